# revision 34
# baseline (speedup 1.0000x reference)
"""Block-sparse attention (block-diagonal mask, full-row softmax) on 8 trn2 cores.

Reference semantics (B=1, H=16, S=4096, D=64, BLOCK=64):
    scores  = (Q @ K^T) / 8                     [S, S] per head
    scores *= blockdiag_mask                    (off-block -> 0, NOT -inf)
    weights = softmax(scores, axis=-1)          (over the FULL row)
    out     = weights @ V

Off-block entries contribute exp(0)=1, so for row q in block b:
    num_q   = sum_{k in b} e_qk v_k - V_bsum(b) + V_total
    denom_q = sum_{k in b} e_qk - 64 + 4096
    out_q   = num_q / denom_q

Sharding: 16 heads over 8 cores -> 2 heads/core, no cross-core comms.

Layout/pipeline (v4):
  - Host pre-transposes Q/K into qkt[64*h + d, pos, side, j]: the two
    heads of a core live in partition halves, so score matmuls are
    64-row quadrant matmuls (tile_position) with NO on-chip transposes.
  - All DMA is large + fully contiguous; f32 -> bf16 casts run on
    GpSimd/ACT/DVE just-in-time per slab (SWDGE cast-DMA is not
    supported by this runtime, and matmul outputs must sit at PSUM bank
    byte-offset 0 - both verified by HW probes).
  - Work unit = (pos, head) pair. Each pair owns one PSUM bank slot of
    a [128, 6, 512] tile: packed scores S^T land as two [64, 64]
    quadrant matmuls at byte 0 (block b in partitions 64b), a batched
    exp (3 pairs per ACT op) emits packed E^T to SBUF, then per block
    EV + all(-1) matmuls overwrite the slot with num|denom [128, 65].
  - denom += 4096 and reciprocal and num*rcp run on DVE at 3-pair
    granularity; the +V_total*rcp term is deferred (it would gate the
    whole pipeline on the full V load) and applied per 8-position slab
    just before its store.
"""

import numpy as np

H, S, D = 16, 4096, 64
HPC = 2  # heads per core
NCORES = 8
CHUNK = 128  # rows per chunk position (2 mask blocks of 64)
NPOS = S // CHUNK  # 32 chunk positions
NPAIR = NPOS * HPC  # 64 (pos, head) pairs; pair p = (c=p//2, h=p%2)
GP = 3  # pairs per exp group
NGRP = (NPAIR + GP - 1) // GP  # 22 (last ragged)
SCALE = 0.125  # 1/sqrt(D)

_CACHE = {}


def _build_bass():
    import concourse.bass as bass
    import concourse.bacc as bacc
    import concourse.tile as tile
    from concourse import mybir

    f32 = mybir.dt.float32
    bf16 = mybir.dt.bfloat16
    EXP = mybir.ActivationFunctionType.Exp
    MULT = mybir.AluOpType.mult
    ADD = mybir.AluOpType.add

    nc = bacc.Bacc(
        "TRN2", target_bir_lowering=False, debug=False, num_devices=NCORES
    )
    # qkt: [128 (64*h + d), pos, side (0=Q,1=K), j]   f32
    qkt_d = nc.dram_tensor("qkt", [128, NPOS, 2, CHUNK], f32, kind="ExternalInput")
    # v: [128 (seq-in-chunk), pos, head, D+1]  (col D is 1.0)  f32
    v_d = nc.dram_tensor("v", [128, NPOS, HPC, D + 1], f32, kind="ExternalInput")
    # out: [128 (seq-in-chunk), pos, head, D]  f32
    o_d = nc.dram_tensor("out", [128, NPOS, HPC, D], f32, kind="ExternalOutput")

    NQS = 8  # qkt slabs (4 positions = 8 pairs each)
    NVS = 4  # v slabs (8 positions = 16 pairs each)

    with tile.TileContext(nc) as tc:
        with (
            tc.tile_pool(name="consts", bufs=1) as consts,
            tc.tile_pool(name="big", bufs=1) as big,
            tc.tile_pool(name="et", bufs=8) as etp,
            tc.tile_pool(name="work", bufs=2) as work,
            tc.tile_pool(name="ps_p", bufs=1, space="PSUM") as ps_p,
        ):
            ones_col = consts.tile([128, 1], bf16, tag="ones_col")
            nc.gpsimd.memset(ones_col, 1.0)
            ones_col_f = consts.tile([128, 1], f32, tag="ones_col_f")
            nc.gpsimd.memset(ones_col_f, 1.0)
            ones_row = consts.tile([1, 128], bf16, tag="ones_row")
            nc.gpsimd.memset(ones_row, 1.0)
            # all(-1): the "-V_bsum" correction as a per-block matmul weight
            neg64 = consts.tile([128, 64], bf16, tag="neg64")
            nc.gpsimd.memset(neg64, -1.0)

            # Two 4-bank PSUM pools: A holds packed scores (freed by the
            # exp read), B holds num|denom (freed by the SBUF evacuation).
            # Each slot's in-PSUM lifetime is just 2 dependency hops; the
            # long quads->exp->EV chain slack lives in SBUF (et tiles).
            psA = ps_p.tile([128, 4, 512], f32, tag="psA")
            psB = ps_p.tile([128, 3, 512], f32, tag="psB")
            # dedicated bank for the V_total accumulator; the replication
            # target borrows a B bank exactly once at wrap time
            vt_ps = ps_p.tile([1, 2, HPC, D + 1], f32, tag="vt_ps")

            qktf = big.tile([128, NPOS, 2, CHUNK], f32, tag="qktf")
            qktb = big.tile([128, NPOS, 2, CHUNK], bf16, tag="qktb")
            vf = big.tile([128, NPOS, HPC, D + 1], f32, tag="vf")
            vhb = big.tile([128, NPOS, HPC, D + 1], bf16, tag="vhb")
            oh = big.tile([128, NPOS, HPC, D], f32, tag="oh")
            pob = big.tile([128, NPAIR, D + 1], f32, tag="pob")  # num|denom
            rcps = big.tile([128, NPAIR], f32, tag="rcps")  # col = 2*pos+h
            vtxb = consts.tile([1, HPC, D + 1], bf16, tag="vtxb")
            vtot2 = big.tile([128, HPC, D + 1], f32, tag="vtot2")
            vtot4 = big.tile([128, 4, D + 1], f32, tag="vtot4")  # h0 h1 h0 h1

            # HAM warm-up: ~200 back-to-back matmuls into the vt bank
            # while the loads stream in. The PE clock-gate un-throttles
            # after ~3.4us of sustained busy (1.2 -> 2.4 GHz) and the
            # steady-state gaps are too short to re-throttle; the real
            # vt chain's start=True overwrites this garbage.
            for _ in range(200):
                nc.tensor.matmul(
                    vt_ps[:, 0, 0, 0:64],
                    ones_col,
                    neg64,
                    start=True,
                    stop=True,
                )

            # ---- loads: interleaved HWDGE f32 slabs ----
            QSL = [(0, 2), (2, 2), (4, 4), (8, 4), (12, 4), (16, 4),
                   (20, 4), (24, 4), (28, 4)]

            def qslab(s, eng=None):
                a, n = QSL[s]
                sl = slice(a, a + n)
                e = eng or nc.sync
                e.dma_start(out=qktf[:, sl, :, :], in_=qkt_d[:, sl, :, :])

            def vslab(s, eng=None):
                sl = slice(8 * s, 8 * (s + 1))
                e = eng or nc.sync
                e.dma_start(out=vf[:, sl, :, :], in_=v_d[:, sl, :, :])

            qslab(0)
            qslab(1)
            vslab(0)
            qslab(2)
            vslab(1)
            qslab(3)
            vslab(2)
            qslab(4)
            vslab(3)
            for s in range(5, len(QSL)):
                qslab(s)

            # casts f32 -> bf16 on DVE/ACT (GpSimd is ~3x slower per elem)
            def qcast(s):
                a, n = QSL[s]
                sl = slice(a, a + n)
                if s % 2 == 0:
                    nc.vector.tensor_copy(out=qktb[:, sl, :, :], in_=qktf[:, sl, :, :])
                else:
                    nc.gpsimd.tensor_copy(out=qktb[:, sl, :, :], in_=qktf[:, sl, :, :])

            def vcast(s):
                sl = slice(8 * s, 8 * (s + 1))
                if s % 2 == 0:
                    nc.vector.tensor_copy(out=vhb[:, sl, :, :], in_=vf[:, sl, :, :])
                else:
                    nc.scalar.copy(out=vhb[:, sl, :, :], in_=vf[:, sl, :, :])

            def scale_slab(s):
                """(num + V_total) / denom for 16 pairs at once, then store.

                vtot2 col D is 4096, so one add handles numerator V_total
                AND the denominator's off-block +4096 in a single op.
                """
                psl = slice(8 * s, 8 * (s + 1))
                ps2 = slice(16 * s, 16 * (s + 1))
                po_s = pob[:, ps2, :].rearrange("p (a b) c -> p a b c", a=8)
                vt_b = vtot2.unsqueeze(1).broadcast_to([128, 8, HPC, D + 1])
                nc.vector.tensor_add(po_s, po_s, vt_b)
                nc.vector.reciprocal(rcps[:, ps2], pob[:, ps2, D])
                rcp_b = rcps[:, ps2].unsqueeze(2).broadcast_to([128, 16, D])
                nc.vector.tensor_tensor(
                    out=oh_ph[:, ps2, :],
                    in0=pob[:, ps2, 0:D],
                    in1=rcp_b,
                    op=MULT,
                )
                nc.scalar.dma_start(out=o_d[:, psl, :, :], in_=oh[:, psl, :, :])

            def scale_half(u):
                # pairs >= 48 had V_total folded into their evacuation
                psl = slice(4 * u, 4 * (u + 1))
                ps2 = slice(8 * u, 8 * (u + 1))
                nc.vector.reciprocal(rcps[:, ps2], pob[:, ps2, D])
                rcp_b = rcps[:, ps2].unsqueeze(2).broadcast_to([128, 8, D])
                nc.vector.tensor_tensor(
                    out=oh_ph[:, ps2, :],
                    in0=pob[:, ps2, 0:D],
                    in1=rcp_b,
                    op=MULT,
                )
                nc.scalar.dma_start(out=o_d[:, psl, :, :], in_=oh[:, psl, :, :])

            def vt_mms(s):
                """V_total partial sums for one v slab: accumulating f32
                matmuls straight from the UNCAST V (no cast dependency),
                into the dedicated vt bank."""
                for m in range(4 * s, 4 * (s + 1)):
                    nc.tensor.matmul(
                        vt_ps,
                        ones_col,
                        vhb[:, 2 * m : 2 * (m + 1), :, :],
                        start=(m == 0),
                        stop=(m == 15),
                    )

            def vt_wrap():
                """Fold pos-parity, replicate across partitions via a
                rank-1 matmul, stash as [128, h, D+1] in SBUF."""
                vt4 = work.tile([1, 2, HPC, D + 1], f32, tag="vt4")
                nc.scalar.copy(out=vt4, in_=vt_ps)
                nc.vector.tensor_add(vtxb, vt4[:, 0, :, :], vt4[:, 1, :, :])
                vtsp = psB[:, 1, 0 : HPC * (D + 1)]  # one-shot B borrow
                nc.tensor.matmul(
                    vtsp,
                    ones_row,
                    vtxb.rearrange("a b c -> a (b c)"),
                    start=True,
                    stop=True,
                )
                nc.vector.tensor_copy(
                    out=vtot2, in_=vtsp.rearrange("p (b c) -> p b c", b=HPC)
                )
                nc.vector.tensor_copy(
                    out=vtot4.rearrange("p (a b) c -> p a b c", a=2),
                    in_=vtsp.rearrange("p (b c) -> p b c", b=HPC)
                    .unsqueeze(1)
                    .broadcast_to([128, 2, HPC, D + 1]),
                )

            oh_ph = oh.rearrange("p c h d -> p (c h) d")  # col = pair idx
            qc_done = 0
            vc_done = 0
            ets = {}
            GPF = 2   # pairs per front group (exp batch); A depth = 2 groups
            NGF = NPAIR // GPF  # 32
            LAGP = 8  # pairs of slack between scores and EV

            def front(g):
                """casts + packed-score quadrant matmuls + exp (2 pairs)."""
                nonlocal qc_done, vc_done
                p0 = GPF * g
                t0 = p0 % 4
                while qc_done < len(QSL) and 2 * (QSL[qc_done][0]) < p0 + GPF:
                    qcast(qc_done)
                    qc_done += 1
                ahead = {2: 8, 3: 16}.get(vc_done, 0)
                while vc_done < NVS and 16 * vc_done < p0 + GPF + ahead:
                    vcast(vc_done)
                    vc_done += 1
                    ahead = {2: 8, 3: 16}.get(vc_done, 0)
                for dp in range(GPF):
                    p = p0 + dp
                    c, h = p // 2, p % 2
                    hp = slice(64 * h, 64 * (h + 1))
                    for b in range(2):
                        bp = slice(64 * b, 64 * (b + 1))
                        nc.tensor.matmul(
                            psA[bp, t0 + dp, 0:64],
                            qktb[hp, c, 1, bp],
                            qktb[hp, c, 0, bp],
                            start=True,
                            stop=True,
                        )
                et = etp.tile([128, GPF, 64], bf16, tag="et", name=f"et{g}")
                nc.scalar.activation(
                    out=et,
                    in_=psA[:, t0 : t0 + GPF, 0:64],
                    func=EXP,
                    scale=SCALE,
                )
                ets[g] = et

            def back(bu):
                """EV + all(-1) matmuls for one 3-pair unit; evacuate."""
                p0 = 3 * bu
                npr = min(3, NPAIR - p0)
                for j in range(npr):
                    p = p0 + j
                    c, h = p // 2, p % 2
                    et = ets[p // GPF]
                    dp = p % GPF
                    for b in range(2):
                        bp = slice(64 * b, 64 * (b + 1))
                        nc.tensor.matmul(
                            psB[bp, j, 0 : D + 1],
                            et[bp, dp, :],
                            vhb[bp, c, h, :],
                            start=True,
                            stop=False,
                        )
                        nc.tensor.matmul(
                            psB[bp, j, 0 : D + 1],
                            neg64[bp, :],
                            vhb[bp, c, h, :],
                            start=False,
                            stop=True,
                        )
                    if p % GPF == GPF - 1 or p == NPAIR - 1:
                        ets.pop(p // GPF)
                if bu >= 16:
                    # slab 3: fold the +V_total into the evacuation itself
                    h0 = p0 % 2
                    nc.vector.tensor_tensor(
                        out=pob[:, p0 : p0 + npr, :],
                        in0=psB[:, 0:npr, 0 : D + 1],
                        in1=vtot4[:, h0 : h0 + npr, :],
                        op=ADD,
                    )
                elif bu % 2 == 0:
                    nc.vector.tensor_copy(
                        out=pob[:, p0 : p0 + npr, :], in_=psB[:, 0:npr, 0 : D + 1]
                    )
                else:
                    nc.scalar.copy(
                        out=pob[:, p0 : p0 + npr, :], in_=psB[:, 0:npr, 0 : D + 1]
                    )

            # ---- main loop: EV units trail the score fronts by LAGP
            # pairs so every cross-engine handoff has real slack ----
            NBU = (NPAIR + 2) // 3  # 22
            bu_done = 0
            vt_done = 0
            wrapped = False
            sc_done = 0
            for g in range(NGF):
                front(g)
                while bu_done < NBU and 3 * bu_done + 3 <= GPF * (g + 1) - LAGP:
                    back(bu_done)
                    bu_done += 1
                while vt_done < NVS and GPF * (g + 1) > 9 * (vt_done + 1):
                    vt_mms(vt_done)
                    vt_done += 1
                if vt_done == NVS and not wrapped:
                    vt_wrap()
                    wrapped = True
                while wrapped and sc_done < 3 and 3 * bu_done >= 16 * (sc_done + 1):
                    scale_slab(sc_done)
                    sc_done += 1
            while bu_done < NBU:
                back(bu_done)
                bu_done += 1
                while wrapped and sc_done < 3 and 3 * bu_done >= 16 * (sc_done + 1):
                    scale_slab(sc_done)
                    sc_done += 1
            scale_half(6)
            scale_half(7)

    nc.compile()
    return nc


def _get_compiled():
    if "nc" not in _CACHE:
        _CACHE["nc"] = _build_bass()
    return _CACHE["nc"]


def make_in_maps(query, key, value):
    q = np.asarray(query).reshape(H, S, D).astype(np.float32)
    k = np.asarray(key).reshape(H, S, D).astype(np.float32)
    v = np.asarray(value).reshape(H, S, D).astype(np.float32)
    in_maps = []
    for i in range(NCORES):
        sl = slice(i * HPC, (i + 1) * HPC)
        # [2, 4096, 64] -> [2, 64, 4096] -> [128, 32, 128] (head-major rows)
        qt = q[sl].transpose(0, 2, 1).reshape(128, NPOS, CHUNK)
        kt = k[sl].transpose(0, 2, 1).reshape(128, NPOS, CHUNK)
        qkt = np.ascontiguousarray(np.stack([qt, kt], axis=2))  # [128,32,2,128]
        # [2, 4096, 64] -> [128 (seq%128), 32 (pos), 2 (head), 64]
        vr = v[sl].reshape(HPC, NPOS, CHUNK, D).transpose(2, 1, 0, 3)
        vp = np.concatenate(
            [vr, np.ones((CHUNK, NPOS, HPC, 1), dtype=np.float32)], axis=3
        )
        in_maps.append({"qkt": qkt, "v": np.ascontiguousarray(vp)})
    return in_maps


def run_spmd(in_maps, **kwargs):
    from concourse.bass_utils import run_bass_kernel_spmd

    nc = _get_compiled()
    return run_bass_kernel_spmd(nc, in_maps, core_ids=list(range(NCORES)), **kwargs)


def assemble(res):
    outs = []
    for i in range(NCORES):
        o = res.results[i]["out"]  # [128, 32, 2, 64]
        outs.append(o.transpose(2, 1, 0, 3).reshape(HPC, S, D))
    return np.concatenate(outs, axis=0).reshape(1, H, S, D).astype(np.float32)


def kernel(query: np.ndarray, key: np.ndarray, value: np.ndarray) -> np.ndarray:
    return assemble(run_spmd(make_in_maps(query, key, value)))


# revision 35
# speedup vs baseline: 1.1182x; 1.1182x over previous
"""Block-sparse attention (block-diagonal mask, full-row softmax) on 8 trn2 cores.

Reference semantics (B=1, H=16, S=4096, D=64, BLOCK=64):
    scores  = (Q @ K^T) / 8                     [S, S] per head
    scores *= blockdiag_mask                    (off-block -> 0, NOT -inf)
    weights = softmax(scores, axis=-1)          (over the FULL row)
    out     = weights @ V

Off-block entries contribute exp(0)=1, so for row q in block b:
    num_q   = sum_{k in b} e_qk v_k - V_bsum(b) + V_total
    denom_q = sum_{k in b} e_qk - 64 + 4096
    out_q   = num_q / denom_q

Sharding: 16 heads over 8 cores -> 2 heads/core, no cross-core comms.

Layout/pipeline (v4):
  - Host pre-transposes Q/K into qkt[64*h + d, pos, side, j]: the two
    heads of a core live in partition halves, so score matmuls are
    64-row quadrant matmuls (tile_position) with NO on-chip transposes.
  - All DMA is large + fully contiguous; f32 -> bf16 casts run on
    GpSimd/ACT/DVE just-in-time per slab (SWDGE cast-DMA is not
    supported by this runtime, and matmul outputs must sit at PSUM bank
    byte-offset 0 - both verified by HW probes).
  - Work unit = (pos, head) pair. Each pair owns one PSUM bank slot of
    a [128, 6, 512] tile: packed scores S^T land as two [64, 64]
    quadrant matmuls at byte 0 (block b in partitions 64b), a batched
    exp (3 pairs per ACT op) emits packed E^T to SBUF, then per block
    EV + all(-1) matmuls overwrite the slot with num|denom [128, 65].
  - denom += 4096 and reciprocal and num*rcp run on DVE at 3-pair
    granularity; the +V_total*rcp term is deferred (it would gate the
    whole pipeline on the full V load) and applied per 8-position slab
    just before its store.
"""

import numpy as np

H, S, D = 16, 4096, 64
HPC = 2  # heads per core
NCORES = 8
CHUNK = 128  # rows per chunk position (2 mask blocks of 64)
NPOS = S // CHUNK  # 32 chunk positions
NPAIR = NPOS * HPC  # 64 (pos, head) pairs; pair p = (c=p//2, h=p%2)
GP = 3  # pairs per exp group
NGRP = (NPAIR + GP - 1) // GP  # 22 (last ragged)
SCALE = 0.125  # 1/sqrt(D)

_CACHE = {}


def _build_bass():
    import concourse.bass as bass
    import concourse.bacc as bacc
    import concourse.tile as tile
    from concourse import mybir

    f32 = mybir.dt.float32
    bf16 = mybir.dt.bfloat16
    EXP = mybir.ActivationFunctionType.Exp
    MULT = mybir.AluOpType.mult
    ADD = mybir.AluOpType.add

    nc = bacc.Bacc(
        "TRN2", target_bir_lowering=False, debug=False, num_devices=NCORES
    )
    # qkt: [128 (64*h + d), pos, side (0=Q,1=K), j]   f32
    qkt_d = nc.dram_tensor("qkt", [128, NPOS, 2, CHUNK], f32, kind="ExternalInput")
    # v: [128 (seq-in-chunk), pos, head, D+1]  (col D is 1.0)  f32
    v_d = nc.dram_tensor("v", [128, NPOS, HPC, D + 1], f32, kind="ExternalInput")
    # out: [128 (seq-in-chunk), pos, head, D]  f32
    o_d = nc.dram_tensor("out", [128, NPOS, HPC, D], f32, kind="ExternalOutput")

    NQS = 8  # qkt slabs (4 positions = 8 pairs each)
    NVS = 4  # v slabs (8 positions = 16 pairs each)

    with tile.TileContext(nc) as tc:
        with (
            tc.tile_pool(name="consts", bufs=1) as consts,
            tc.tile_pool(name="big", bufs=1) as big,
            tc.tile_pool(name="et", bufs=8) as etp,
            tc.tile_pool(name="work", bufs=2) as work,
            tc.tile_pool(name="ps_p", bufs=1, space="PSUM") as ps_p,
        ):
            ones_col = consts.tile([128, 1], bf16, tag="ones_col")
            nc.gpsimd.memset(ones_col, 1.0)
            ones_col_f = consts.tile([128, 1], f32, tag="ones_col_f")
            nc.gpsimd.memset(ones_col_f, 1.0)
            ones_row = consts.tile([1, 128], bf16, tag="ones_row")
            nc.gpsimd.memset(ones_row, 1.0)
            # all(-1): the "-V_bsum" correction as a per-block matmul weight
            neg64 = consts.tile([128, 64], bf16, tag="neg64")
            nc.gpsimd.memset(neg64, -1.0)

            # Two 4-bank PSUM pools: A holds packed scores (freed by the
            # exp read), B holds num|denom (freed by the SBUF evacuation).
            # Each slot's in-PSUM lifetime is just 2 dependency hops; the
            # long quads->exp->EV chain slack lives in SBUF (et tiles).
            psA = ps_p.tile([128, 4, 512], f32, tag="psA")
            psB = ps_p.tile([128, 3, 512], f32, tag="psB")
            # dedicated bank for the V_total accumulator; the replication
            # target borrows a B bank exactly once at wrap time
            vt_ps = ps_p.tile([1, 2, HPC, D + 1], f32, tag="vt_ps")

            qktf = big.tile([128, NPOS, 2, CHUNK], f32, tag="qktf")
            qktb = big.tile([128, NPOS, 2, CHUNK], bf16, tag="qktb")
            vf = big.tile([128, NPOS, HPC, D + 1], f32, tag="vf")
            vhb = big.tile([128, NPOS, HPC, D + 1], bf16, tag="vhb")
            oh = big.tile([128, NPOS, HPC, D], f32, tag="oh")
            pob = big.tile([128, NPAIR, D + 1], f32, tag="pob")  # num|denom
            rcps = big.tile([128, NPAIR], f32, tag="rcps")  # col = 2*pos+h
            vtxb = consts.tile([1, HPC, D + 1], bf16, tag="vtxb")
            vtot2 = big.tile([128, HPC, D + 1], f32, tag="vtot2")
            vtot4 = big.tile([128, 4, D + 1], f32, tag="vtot4")  # h0 h1 h0 h1

            # HAM warm-up: ~200 back-to-back matmuls into the vt bank
            # while the loads stream in. The PE clock-gate un-throttles
            # after ~3.4us of sustained busy (1.2 -> 2.4 GHz) and the
            # steady-state gaps are too short to re-throttle; the real
            # vt chain's start=True overwrites this garbage.
            for _ in range(90):
                nc.tensor.matmul(
                    vt_ps[:, 0, 0, 0:64],
                    ones_col,
                    neg64,
                    start=True,
                    stop=True,
                )

            # ---- loads: interleaved HWDGE f32 slabs ----
            QSL = [(0, 2), (2, 2), (4, 4), (8, 4), (12, 4), (16, 4),
                   (20, 4), (24, 4), (28, 4)]

            def qslab(s, eng=None):
                a, n = QSL[s]
                sl = slice(a, a + n)
                e = eng or nc.sync
                e.dma_start(out=qktf[:, sl, :, :], in_=qkt_d[:, sl, :, :])

            def vslab(s, eng=None):
                sl = slice(8 * s, 8 * (s + 1))
                e = eng or nc.sync
                e.dma_start(out=vf[:, sl, :, :], in_=v_d[:, sl, :, :])

            qslab(0)
            qslab(1)
            vslab(0)
            qslab(2)
            vslab(1)
            qslab(3)
            vslab(2)
            qslab(4)
            vslab(3)
            for s in range(5, len(QSL)):
                qslab(s)

            # casts f32 -> bf16 on DVE/ACT (GpSimd is ~3x slower per elem)
            def qcast(s):
                a, n = QSL[s]
                sl = slice(a, a + n)
                if s % 2 == 0:
                    nc.vector.tensor_copy(out=qktb[:, sl, :, :], in_=qktf[:, sl, :, :])
                else:
                    nc.gpsimd.tensor_copy(out=qktb[:, sl, :, :], in_=qktf[:, sl, :, :])

            def vcast(s):
                sl = slice(8 * s, 8 * (s + 1))
                if s % 2 == 0:
                    nc.vector.tensor_copy(out=vhb[:, sl, :, :], in_=vf[:, sl, :, :])
                else:
                    nc.scalar.copy(out=vhb[:, sl, :, :], in_=vf[:, sl, :, :])

            def scale_slab(s):
                """(num + V_total) / denom for 16 pairs at once, then store.

                vtot2 col D is 4096, so one add handles numerator V_total
                AND the denominator's off-block +4096 in a single op.
                """
                psl = slice(8 * s, 8 * (s + 1))
                ps2 = slice(16 * s, 16 * (s + 1))
                po_s = pob[:, ps2, :].rearrange("p (a b) c -> p a b c", a=8)
                vt_b = vtot2.unsqueeze(1).broadcast_to([128, 8, HPC, D + 1])
                nc.vector.tensor_add(po_s, po_s, vt_b)
                nc.vector.reciprocal(rcps[:, ps2], pob[:, ps2, D])
                rcp_b = rcps[:, ps2].unsqueeze(2).broadcast_to([128, 16, D])
                nc.vector.tensor_tensor(
                    out=oh_ph[:, ps2, :],
                    in0=pob[:, ps2, 0:D],
                    in1=rcp_b,
                    op=MULT,
                )
                nc.scalar.dma_start(out=o_d[:, psl, :, :], in_=oh[:, psl, :, :])

            def scale_half(u):
                # pairs >= 48 had V_total folded into their evacuation
                psl = slice(4 * u, 4 * (u + 1))
                ps2 = slice(8 * u, 8 * (u + 1))
                nc.vector.reciprocal(rcps[:, ps2], pob[:, ps2, D])
                rcp_b = rcps[:, ps2].unsqueeze(2).broadcast_to([128, 8, D])
                nc.vector.tensor_tensor(
                    out=oh_ph[:, ps2, :],
                    in0=pob[:, ps2, 0:D],
                    in1=rcp_b,
                    op=MULT,
                )
                nc.scalar.dma_start(out=o_d[:, psl, :, :], in_=oh[:, psl, :, :])

            def vt_mms(s):
                """V_total partial sums for one v slab: accumulating f32
                matmuls straight from the UNCAST V (no cast dependency),
                into the dedicated vt bank."""
                for m in range(4 * s, 4 * (s + 1)):
                    nc.tensor.matmul(
                        vt_ps,
                        ones_col,
                        vhb[:, 2 * m : 2 * (m + 1), :, :],
                        start=(m == 0),
                        stop=(m == 15),
                    )

            def vt_wrap():
                """Fold pos-parity, replicate across partitions via a
                rank-1 matmul, stash as [128, h, D+1] in SBUF."""
                vt4 = work.tile([1, 2, HPC, D + 1], f32, tag="vt4")
                nc.scalar.copy(out=vt4, in_=vt_ps)
                nc.vector.tensor_add(vtxb, vt4[:, 0, :, :], vt4[:, 1, :, :])
                vtsp = psB[:, 1, 0 : HPC * (D + 1)]  # one-shot B borrow
                nc.tensor.matmul(
                    vtsp,
                    ones_row,
                    vtxb.rearrange("a b c -> a (b c)"),
                    start=True,
                    stop=True,
                )
                nc.vector.tensor_copy(
                    out=vtot2, in_=vtsp.rearrange("p (b c) -> p b c", b=HPC)
                )
                nc.vector.tensor_copy(
                    out=vtot4.rearrange("p (a b) c -> p a b c", a=2),
                    in_=vtsp.rearrange("p (b c) -> p b c", b=HPC)
                    .unsqueeze(1)
                    .broadcast_to([128, 2, HPC, D + 1]),
                )

            oh_ph = oh.rearrange("p c h d -> p (c h) d")  # col = pair idx
            qc_done = 0
            vc_done = 0
            ets = {}
            GPF = 2   # pairs per front group (exp batch); A depth = 2 groups
            NGF = NPAIR // GPF  # 32
            LAGP = 8  # pairs of slack between scores and EV

            def front(g):
                """casts + packed-score quadrant matmuls + exp (2 pairs)."""
                nonlocal qc_done, vc_done
                p0 = GPF * g
                t0 = p0 % 4
                while qc_done < len(QSL) and 2 * (QSL[qc_done][0]) < p0 + GPF:
                    qcast(qc_done)
                    qc_done += 1
                ahead = {2: 8, 3: 16}.get(vc_done, 0)
                while vc_done < NVS and 16 * vc_done < p0 + GPF + ahead:
                    vcast(vc_done)
                    vc_done += 1
                    ahead = {2: 8, 3: 16}.get(vc_done, 0)
                for dp in range(GPF):
                    p = p0 + dp
                    c, h = p // 2, p % 2
                    hp = slice(64 * h, 64 * (h + 1))
                    for b in range(2):
                        bp = slice(64 * b, 64 * (b + 1))
                        nc.tensor.matmul(
                            psA[bp, t0 + dp, 0:64],
                            qktb[hp, c, 1, bp],
                            qktb[hp, c, 0, bp],
                            start=True,
                            stop=True,
                        )
                et = etp.tile([128, GPF, 64], bf16, tag="et", name=f"et{g}")
                nc.scalar.activation(
                    out=et,
                    in_=psA[:, t0 : t0 + GPF, 0:64],
                    func=EXP,
                    scale=SCALE,
                )
                ets[g] = et

            def back(bu):
                """EV + all(-1) matmuls for one 3-pair unit; evacuate."""
                p0 = 3 * bu
                npr = min(3, NPAIR - p0)
                for j in range(npr):
                    p = p0 + j
                    c, h = p // 2, p % 2
                    et = ets[p // GPF]
                    dp = p % GPF
                    for b in range(2):
                        bp = slice(64 * b, 64 * (b + 1))
                        nc.tensor.matmul(
                            psB[bp, j, 0 : D + 1],
                            et[bp, dp, :],
                            vhb[bp, c, h, :],
                            start=True,
                            stop=False,
                        )
                        nc.tensor.matmul(
                            psB[bp, j, 0 : D + 1],
                            neg64[bp, :],
                            vhb[bp, c, h, :],
                            start=False,
                            stop=True,
                        )
                    if p % GPF == GPF - 1 or p == NPAIR - 1:
                        ets.pop(p // GPF)
                if bu >= 16:
                    # slab 3: fold the +V_total into the evacuation itself
                    h0 = p0 % 2
                    nc.vector.tensor_tensor(
                        out=pob[:, p0 : p0 + npr, :],
                        in0=psB[:, 0:npr, 0 : D + 1],
                        in1=vtot4[:, h0 : h0 + npr, :],
                        op=ADD,
                    )
                elif bu % 2 == 0:
                    nc.vector.tensor_copy(
                        out=pob[:, p0 : p0 + npr, :], in_=psB[:, 0:npr, 0 : D + 1]
                    )
                else:
                    nc.scalar.copy(
                        out=pob[:, p0 : p0 + npr, :], in_=psB[:, 0:npr, 0 : D + 1]
                    )

            # ---- main loop: EV units trail the score fronts by LAGP
            # pairs so every cross-engine handoff has real slack ----
            NBU = (NPAIR + 2) // 3  # 22
            bu_done = 0
            vt_done = 0
            wrapped = False
            sc_done = 0
            for g in range(NGF):
                front(g)
                while bu_done < NBU and 3 * bu_done + 3 <= GPF * (g + 1) - LAGP:
                    back(bu_done)
                    bu_done += 1
                while vt_done < NVS and GPF * (g + 1) > 9 * (vt_done + 1):
                    vt_mms(vt_done)
                    vt_done += 1
                if vt_done == NVS and not wrapped:
                    vt_wrap()
                    wrapped = True
                while wrapped and sc_done < 3 and 3 * bu_done >= 16 * (sc_done + 1):
                    scale_slab(sc_done)
                    sc_done += 1
            while bu_done < NBU:
                back(bu_done)
                bu_done += 1
                while wrapped and sc_done < 3 and 3 * bu_done >= 16 * (sc_done + 1):
                    scale_slab(sc_done)
                    sc_done += 1
            scale_half(6)
            scale_half(7)

    nc.compile()
    return nc


def _get_compiled():
    if "nc" not in _CACHE:
        _CACHE["nc"] = _build_bass()
    return _CACHE["nc"]


def make_in_maps(query, key, value):
    q = np.asarray(query).reshape(H, S, D).astype(np.float32)
    k = np.asarray(key).reshape(H, S, D).astype(np.float32)
    v = np.asarray(value).reshape(H, S, D).astype(np.float32)
    in_maps = []
    for i in range(NCORES):
        sl = slice(i * HPC, (i + 1) * HPC)
        # [2, 4096, 64] -> [2, 64, 4096] -> [128, 32, 128] (head-major rows)
        qt = q[sl].transpose(0, 2, 1).reshape(128, NPOS, CHUNK)
        kt = k[sl].transpose(0, 2, 1).reshape(128, NPOS, CHUNK)
        qkt = np.ascontiguousarray(np.stack([qt, kt], axis=2))  # [128,32,2,128]
        # [2, 4096, 64] -> [128 (seq%128), 32 (pos), 2 (head), 64]
        vr = v[sl].reshape(HPC, NPOS, CHUNK, D).transpose(2, 1, 0, 3)
        vp = np.concatenate(
            [vr, np.ones((CHUNK, NPOS, HPC, 1), dtype=np.float32)], axis=3
        )
        in_maps.append({"qkt": qkt, "v": np.ascontiguousarray(vp)})
    return in_maps


def run_spmd(in_maps, **kwargs):
    from concourse.bass_utils import run_bass_kernel_spmd

    nc = _get_compiled()
    return run_bass_kernel_spmd(nc, in_maps, core_ids=list(range(NCORES)), **kwargs)


def assemble(res):
    outs = []
    for i in range(NCORES):
        o = res.results[i]["out"]  # [128, 32, 2, 64]
        outs.append(o.transpose(2, 1, 0, 3).reshape(HPC, S, D))
    return np.concatenate(outs, axis=0).reshape(1, H, S, D).astype(np.float32)


def kernel(query: np.ndarray, key: np.ndarray, value: np.ndarray) -> np.ndarray:
    return assemble(run_spmd(make_in_maps(query, key, value)))
